# revision 27
# baseline (speedup 1.0000x reference)
"""AugGraphConv (per-relation GAT + lang-level softmax) on 8 TRN2 NeuronCores.

v2 — transfer-optimized (the axon tunnel at ~35-80MB/s dominates wall):
  - x is SHARDED: core m receives only rows [m*S,(m+1)*S), quantized to
    per-row int8 (LayerNorm is scale-invariant per row, so no scales ship).
    Stage A (LN + per-relation features) runs on owned rows only; the bf16
    feature tables are AllGathered on-device (5 x 13.7MB over NeuronLink,
    Shared output buffers).
  - ONE u8 input array per core carries [x int8 | src-id u16 bytes |
    dst-slot u8]; gather offsets (i32) / one-hot keys (bf16) are
    upconverted on device. Weights ride in one bf16 array; attention/bias
    rows ship as a [1,1024] f32 row broadcast on device with a K=1 matmul;
    iota/identity are NEFF consts.
  - Output is gelu(...) WITHOUT the +x residual (host adds x_inp in f32),
    int8-quantized with a per-node f32 scale bitcast into the same array.
  - The compiled SPMD executable is CACHED (_CachedExec); repeat calls pay
    only H2D + exec + D2H, not jit re-trace/lowering of the ~15k-inst BIR.
    Donated output buffers are recycled from the previous call.
Compute structure (per core, dst-sharded graph parallel):
  edges binned by (own dst tile, relation) into 128-slot chunks; segment
  softmax without max-subtraction; one-hot scatter-add via PE matmuls.
"""

import os
import numpy as np
import ml_dtypes
from contextlib import ExitStack

import concourse.bass as bass
import concourse.mybir as mybir
from concourse.bass import IndirectOffsetOnAxis
from concourse.tile import TileContext
from concourse.bass_utils import run_bass_kernel_spmd

N, D, H, R, C = 50000, 128, 8, 5, 16
P = 128
M = 8
NPAD = 50176            # 392 * 128, divisible by M*P
S = NPAD // M           # 6272 rows per core
T = S // P              # 49 owned tiles per core
FD = D + H              # 136: [xw | al]
ARPAD = R * S + 2 * P   # arrel rows incl. pad region (covers idx <= 31432)
F32 = mybir.dt.float32
F16 = mybir.dt.float16
BF16 = mybir.dt.bfloat16
I32 = mybir.dt.int32
I8 = mybir.dt.int8
U16 = mybir.dt.uint16
U8 = mybir.dt.uint8
MAGIC = 12582912.0      # 1.5*2^23: (v+MAGIC)-MAGIC rounds f32 to nearest int
AF = mybir.ActivationFunctionType
ALU = mybir.AluOpType
AX = mybir.AxisListType
NEGM = -30.0            # softmax mask value (exp(-30) ~ 1e-13, negligible)

LAST_RESULTS = None     # test.py reads exec_time_ns / profile from here


def _split_multiwaits(nc):
    """This toolchain's walrus codegen allows only one sem-wait per
    instruction; hoist extra waits into preceding NoOps on the same engine
    (sequencer executes them in program order, so semantics are identical)."""
    n_split = 0
    for _, bbwrap in nc.bb_map.items():
        bb = bbwrap.bb
        out = []
        changed = False
        for inst in list(bb.instructions):
            si = inst.sync_info
            if si is not None and si.on_wait is not None and len(si.on_wait) > 1:
                waits = list(si.on_wait)
                for w in waits[:-1]:
                    out.append(mybir.InstNoOp(
                        name=nc.get_next_instruction_name(),
                        engine=inst.engine, ins=[], outs=[],
                        sync_info=mybir.SyncInfo(on_wait=[w], on_update=[])))
                    n_split += 1
                si.on_wait = waits[-1:]
                inst.sync_info = si
                changed = True
            out.append(inst)
        if changed:
            bb.instructions = out
    return n_split


def _build(K, TOTC):
    nc = bass.Bass(num_devices=M)
    # merged I/O (the axon tunnel has substantial per-array overhead):
    #   wall = [wcat(680) | vcat(40) | wself(128) | wcross(128)] bf16
    #   etab = [x int8 tile-blocks (T*D) | src u16 bytes (2*TOTC) |
    #           dst u8 (TOTC)] per partition
    #   out_q = [int8 q (D) | f32 row-scale bitcast (4)] per row
    XW = 96                                     # packed x bytes per tile row
    TD = T * XW
    WC = R * FD + R * H + 2 * D                 # wall bf16 columns (976)
    WSB = 2 * WC // M                           # wall-shard bytes/partition
    WOFF = -(-(TD + 3 * TOTC) // 4) * 4         # 4B-aligned shard byte offset
    POFF = WOFF + WSB                           # prow f32 byte offset
    ECOLS = POFF + 32
    etab = nc.declare_dram_parameter("etab", [P, ECOLS], U8, isOutput=False)
    out_q = nc.declare_dram_parameter("out_q", [S, D + 4], I8, isOutput=True)

    ident_d = nc.inline_tensor(np.eye(P, dtype=np.float32), name="ident_c")
    iorow_d = nc.inline_tensor(
        np.tile(np.arange(P, dtype=np.float32)[None, :], (P, 1))
        .astype(ml_dtypes.bfloat16), name="iorow_c")

    wl_loc = nc.dram_tensor("wl_loc", [P // M, 2 * WC], U8)
    wl_glob = nc.dram_tensor("wl_glob", [P, 2 * WC], U8, addr_space="Shared")
    featl = [nc.dram_tensor(f"featl{r}", [S, FD], BF16) for r in range(R)]
    featg = [nc.dram_tensor(f"featg{r}", [NPAD, FD], BF16, addr_space="Shared")
             for r in range(R)]
    arrel = nc.dram_tensor("ar_rel", [ARPAD, H], BF16)

    groups = [list(range(M))]

    with TileContext(nc) as tc, ExitStack() as ctx:
        cp = ctx.enter_context(tc.tile_pool(name="const", bufs=1))
        so = ctx.enter_context(tc.tile_pool(name="sown", bufs=1))
        sb = ctx.enter_context(tc.tile_pool(name="sb", bufs=3))
        eb = ctx.enter_context(tc.tile_pool(name="eb", bufs=4))
        lb = ctx.enter_context(tc.tile_pool(name="lb", bufs=2))
        psA = ctx.enter_context(tc.tile_pool(name="psA", bufs=2, space="PSUM"))
        psB = ctx.enter_context(tc.tile_pool(name="psB", bufs=1, space="PSUM"))

        # ---- persistent constants / index arrays ----
        # wall weights are SHARDED across cores inside etab: bounce this
        # core's [P, WSB] slice to DRAM as [P/M, 2*WC] (same flat order),
        # AllGather to the full [P, 2*WC], then re-type to a bf16 tile.
        wsh_s = cp.tile([P, WSB], U8)
        nc.gpsimd.dma_start(out=wsh_s[:], in_=etab[:, WOFF:WOFF + WSB])
        nc.gpsimd.dma_start(out=wl_loc[:], in_=wsh_s[:])
        nc.gpsimd.collective_compute(
            "AllGather", ALU.bypass, replica_groups=groups,
            ins=[wl_loc[:]], outs=[wl_glob[:]])
        wall_s = cp.tile([D, WC], BF16)
        nc.gpsimd.dma_start(out=wall_s[:], in_=wl_glob[:].bitcast(BF16))
        vcat_s = wall_s[:, R * FD:R * FD + R * H]
        wself_s = wall_s[:, R * FD + R * H:R * FD + R * H + D]
        wcross_s = wall_s[:, R * FD + R * H + D:R * FD + R * H + 2 * D]
        iden_s = cp.tile([P, P], F32)
        nc.gpsimd.dma_start(out=iden_s[:], in_=ident_d[:])
        iorow_s = cp.tile([P, P], BF16)
        nc.gpsimd.dma_start(out=iorow_s[:], in_=iorow_d[:])

        etab_s = cp.tile([P, TD + 3 * TOTC], U8)
        nc.gpsimd.dma_start(out=etab_s[:], in_=etab[:, 0:TD + 3 * TOTC])
        srcu_s = etab_s[:, TD:TD + 2 * TOTC].bitcast(U16)  # [P,TOTC] u16 view
        dstu_s = etab_s[:, TD + 2 * TOTC:TD + 3 * TOTC]
        srci_s = cp.tile([P, TOTC], I32)
        nc.vector.tensor_copy(out=srci_s[:], in_=srcu_s)
        dsti_s = cp.tile([P, TOTC], I32)
        nc.vector.tensor_copy(out=dsti_s[:], in_=dstu_s)
        dstb_s = cp.tile([P, TOTC], BF16)
        nc.vector.tensor_copy(out=dstb_s[:], in_=dstu_s)

        # ---- broadcast param row [1,1024] -> [P,1024] via K=1 matmul ----
        # [P,8] f32 region linearizes row-major to the [1,1024] prow row
        prow_s = cp.tile([1, 8 * D], F32)
        nc.gpsimd.dma_start(out=prow_s[:],
                            in_=etab[:, POFF:POFF + 32].bitcast(F32))
        ones_s = cp.tile([1, P], F32)
        nc.vector.memset(ones_s[:], 1.0)
        params_s = cp.tile([P, 8 * D], F32)
        for h in range(8):
            pr_ps = psA.tile([P, P], F32, tag="tp")
            nc.tensor.matmul(out=pr_ps[:], lhsT=ones_s[:],
                             rhs=prow_s[:, h * D:(h + 1) * D],
                             start=True, stop=True)
            nc.vector.tensor_copy(out=params_s[:, h * D:(h + 1) * D],
                                  in_=pr_ps[:])
        asl_s = params_s[:, 0:D]
        adl_s = params_s[:, D:2 * D]
        bl_s = params_s[:, 2 * D:3 * D]
        bw_s = params_s[:, 3 * D:8 * D]

        # zero the arrel pad region (gathers may touch rows >= R*S)
        zero_s = cp.tile([P, H], BF16)
        nc.vector.memset(zero_s[:], 0.0)
        for i in range(2):
            nc.gpsimd.dma_start(out=arrel[R * S + i * P:R * S + (i + 1) * P, :],
                                in_=zero_s[:])

        # ---- Stage A: LN + per-relation features for OWNED nodes only ----
        sown_tiles = []
        for t in range(T):
            # unpack 4x6-bit planar x: section [b0(32)|b1(32)|b2(32)] bytes;
            # feature order is plane-major (host permuted weight rows to match)
            xo = t * XW
            bi = sb.tile([P, XW], I32, tag="bi")
            nc.vector.tensor_copy(out=bi[:], in_=etab_s[:, xo:xo + XW])
            i0, i1, i2 = bi[:, 0:32], bi[:, 32:64], bi[:, 64:96]
            xi = sb.tile([P, D], I32, tag="xi")
            ta = sb.tile([P, 32], I32, tag="ta")
            tb = sb.tile([P, 32], I32, tag="tb")
            nc.vector.tensor_scalar(out=xi[:, 0:32], in0=i0, scalar1=2,
                                    scalar2=None, op0=ALU.logical_shift_right)
            nc.vector.tensor_scalar(out=ta[:], in0=i0, scalar1=3, scalar2=None,
                                    op0=ALU.bitwise_and)
            nc.vector.tensor_scalar(out=ta[:], in0=ta[:], scalar1=4,
                                    scalar2=None, op0=ALU.logical_shift_left)
            nc.vector.tensor_scalar(out=tb[:], in0=i1, scalar1=4, scalar2=None,
                                    op0=ALU.logical_shift_right)
            nc.vector.tensor_tensor(out=xi[:, 32:64], in0=ta[:], in1=tb[:],
                                    op=ALU.bitwise_or)
            nc.vector.tensor_scalar(out=ta[:], in0=i1, scalar1=15, scalar2=None,
                                    op0=ALU.bitwise_and)
            nc.vector.tensor_scalar(out=ta[:], in0=ta[:], scalar1=2,
                                    scalar2=None, op0=ALU.logical_shift_left)
            nc.vector.tensor_scalar(out=tb[:], in0=i2, scalar1=6, scalar2=None,
                                    op0=ALU.logical_shift_right)
            nc.vector.tensor_tensor(out=xi[:, 64:96], in0=ta[:], in1=tb[:],
                                    op=ALU.bitwise_or)
            nc.vector.tensor_scalar(out=xi[:, 96:128], in0=i2, scalar1=63,
                                    scalar2=None, op0=ALU.bitwise_and)
            xt = sb.tile([P, D], F32, tag="xt")
            nc.vector.tensor_scalar(out=xt[:], in0=xi[:], scalar1=32,
                                    scalar2=None, op0=ALU.subtract)
            mu = sb.tile([P, 1], F32, tag="mu")
            nc.vector.tensor_reduce(out=mu[:], in_=xt[:], axis=AX.X, op=ALU.add)
            nc.vector.tensor_scalar_mul(out=mu[:], in0=mu[:], scalar1=1.0 / D)
            xc = sb.tile([P, D], F32, tag="xc")
            nc.vector.tensor_scalar(out=xc[:], in0=xt[:], scalar1=mu[:],
                                    scalar2=None, op0=ALU.subtract)
            sq = sb.tile([P, D], F32, tag="sq")
            nc.scalar.activation(out=sq[:], in_=xc[:], func=AF.Square)
            var = sb.tile([P, 1], F32, tag="var")
            nc.vector.tensor_reduce(out=var[:], in_=sq[:], axis=AX.X, op=ALU.add)
            nc.vector.tensor_scalar(out=var[:], in0=var[:], scalar1=1.0 / D,
                                    scalar2=1e-5, op0=ALU.mult, op1=ALU.add)
            sd = sb.tile([P, 1], F32, tag="sd")
            nc.scalar.activation(out=sd[:], in_=var[:], func=AF.Sqrt)
            rs = sb.tile([P, 1], F32, tag="rs")
            nc.vector.reciprocal(out=rs[:], in_=sd[:])
            xn = sb.tile([P, D], F32, tag="xn")
            nc.vector.tensor_scalar_mul(out=xn[:], in0=xc[:], scalar1=rs[:])
            tp = psA.tile([P, P], F32, tag="tp")
            nc.tensor.transpose(out=tp[:], in_=xn[:], identity=iden_s[:])
            xnT = sb.tile([P, P], BF16, tag="xnT")
            nc.vector.tensor_copy(out=xnT[:], in_=tp[:])
            for r in range(R):
                fm = psA.tile([P, FD], F32, tag="fm")
                nc.tensor.matmul(out=fm[:], lhsT=xnT[:],
                                 rhs=wall_s[:, r * FD:(r + 1) * FD],
                                 start=True, stop=True)
                fc = sb.tile([P, FD], BF16, tag="fc")
                nc.vector.tensor_copy(out=fc[:], in_=fm[:])
                nc.gpsimd.dma_start(out=featl[r][t * P:(t + 1) * P, :], in_=fc[:])
            am = psA.tile([P, FD], F32, tag="fm")
            nc.tensor.matmul(out=am[:, :R * H], lhsT=xnT[:], rhs=vcat_s,
                             start=True, stop=True)
            ac = sb.tile([P, R * H], BF16, tag="ac")
            nc.vector.tensor_copy(out=ac[:], in_=am[:, :R * H])
            for r in range(R):
                nc.gpsimd.dma_start(
                    out=arrel[r * S + t * P:r * S + (t + 1) * P, :],
                    in_=ac[:, r * H:(r + 1) * H])
            sm_ = psA.tile([P, FD], F32, tag="fm")
            nc.tensor.matmul(out=sm_[:, :D], lhsT=xnT[:], rhs=wself_s,
                             start=True, stop=True)
            sc = so.tile([P, D], F32, tag=f"sown{t}")
            nc.vector.tensor_copy(out=sc[:], in_=sm_[:, :D])
            sown_tiles.append(sc)

        # ---- AllGather per-relation feature tables across the 8 cores ----
        for r in range(R):
            nc.gpsimd.collective_compute(
                "AllGather", ALU.bypass, replica_groups=groups,
                ins=[featl[r][:]], outs=[featg[r][:]])

        # ---- Stage B: edge aggregation + lang softmax, per owned tile ----
        c = 0
        for t in range(T):
            maskp = lb.tile([P, (R + 1) * H], F32, tag="maskp")
            nc.vector.memset(maskp[:, 0:H], 1.0)
            vts = []
            for r in range(R):
                Kt = K[t][r]
                nd_ps = psB.tile([P, FD], F32, tag="nd")
                num_ps = nd_ps[:, 0:D]
                den_ps = nd_ps[:, D:FD]
                for k in range(Kt):
                    G = eb.tile([P, FD], BF16, tag="G")
                    nc.gpsimd.indirect_dma_start(
                        out=G[:], out_offset=None, in_=featg[r][:],
                        in_offset=IndirectOffsetOnAxis(ap=srci_s[:, c:c + 1], axis=0))
                    ari = eb.tile([P, 1], I32, tag="ari")
                    nc.vector.tensor_scalar(out=ari[:], in0=dsti_s[:, c:c + 1],
                                            scalar1=r * S + t * P, scalar2=None,
                                            op0=ALU.add)
                    Aar = eb.tile([P, H], BF16, tag="Aar")
                    nc.gpsimd.indirect_dma_start(
                        out=Aar[:], out_offset=None, in_=arrel[:],
                        in_offset=IndirectOffsetOnAxis(ap=ari[:], axis=0))
                    lg = eb.tile([P, H], F32, tag="lg")
                    nc.vector.tensor_add(out=lg[:], in0=G[:, D:FD], in1=Aar[:])
                    l2 = eb.tile([P, H], F32, tag="l2")
                    nc.vector.tensor_scalar_mul(out=l2[:], in0=lg[:], scalar1=0.2)
                    lr = eb.tile([P, H], F32, tag="lr")
                    nc.vector.tensor_tensor(out=lr[:], in0=lg[:], in1=l2[:],
                                            op=ALU.max)
                    wb = eb.tile([P, H], BF16, tag="wb")
                    nc.scalar.activation(out=wb[:], in_=lr[:], func=AF.Exp)
                    V = eb.tile([P, FD], BF16, tag="V")
                    nc.vector.tensor_copy(out=V[:, D:FD], in_=wb[:])
                    Sm = eb.tile([P, P], BF16, tag="Sm")
                    nc.vector.tensor_tensor(
                        out=Sm[:], in0=dstb_s[:, c:c + 1].to_broadcast([P, P]),
                        in1=iorow_s[:], op=ALU.is_equal)
                    nc.vector.tensor_tensor(
                        out=V[:, 0:D].rearrange("p (h c) -> p h c", c=C),
                        in0=G[:, 0:D].rearrange("p (h c) -> p h c", c=C),
                        in1=wb[:, :, None].to_broadcast([P, H, C]),
                        op=ALU.mult)
                    nc.tensor.matmul(out=nd_ps[:], lhsT=Sm[:], rhs=V[:],
                                     start=(k == 0), stop=(k == Kt - 1))
                    c += 1
                den1 = eb.tile([P, H], F32, tag="den1")
                nc.vector.tensor_scalar_max(out=den1[:], in0=den_ps[:],
                                            scalar1=1e-6)
                rec = eb.tile([P, H], F32, tag="rec")
                nc.vector.reciprocal(out=rec[:], in_=den1[:])
                nc.vector.tensor_scalar(
                    out=maskp[:, (r + 1) * H:(r + 2) * H], in0=den_ps[:],
                    scalar1=0.0, scalar2=None, op0=ALU.is_gt)
                O = eb.tile([P, D], F32, tag="O")
                nc.vector.tensor_tensor(
                    out=O[:].rearrange("p (h c) -> p h c", c=C),
                    in0=num_ps[:].rearrange("p (h c) -> p h c", c=C),
                    in1=rec[:, :, None].to_broadcast([P, H, C]),
                    op=ALU.mult)
                nc.vector.tensor_add(out=O[:], in0=O[:],
                                     in1=bw_s[:, r * D:(r + 1) * D])
                g = eb.tile([P, D], F32, tag="g")
                nc.scalar.activation(out=g[:], in_=O[:], func=AF.Gelu)
                tpb = psA.tile([P, P], F32, tag="tp")
                nc.tensor.transpose(out=tpb[:], in_=g[:], identity=iden_s[:])
                gT = eb.tile([P, P], BF16, tag="gT")
                nc.vector.tensor_copy(out=gT[:], in_=tpb[:])
                v_ps = psB.tile([P, D], F32, tag="vps")
                nc.tensor.matmul(out=v_ps[:], lhsT=gT[:], rhs=wcross_s,
                                 start=True, stop=True)
                vr = lb.tile([P, D], F32, tag=f"v{r + 1}")
                nc.vector.tensor_copy(out=vr[:], in_=v_ps[:])
                vts.append(vr)

            # lang-level GAT over 6 feature rows for this tile
            v0 = sown_tiles[t]
            vall = [v0] + vts
            alp = lb.tile([P, (R + 1) * H], F32, tag="alp")
            tmp = lb.tile([P, D], F32, tag="ltmp")
            for kk in range(R + 1):
                nc.vector.tensor_tensor(out=tmp[:], in0=vall[kk][:],
                                        in1=asl_s, op=ALU.mult)
                nc.vector.tensor_reduce(
                    out=alp[:, kk * H:(kk + 1) * H],
                    in_=tmp[:].rearrange("p (h c) -> p h c", c=C),
                    axis=AX.X, op=ALU.add)
            arl = lb.tile([P, H], F32, tag="arl")
            nc.vector.tensor_tensor(out=tmp[:], in0=v0[:], in1=adl_s,
                                    op=ALU.mult)
            nc.vector.tensor_reduce(
                out=arl[:], in_=tmp[:].rearrange("p (h c) -> p h c", c=C),
                axis=AX.X, op=ALU.add)
            lgp = lb.tile([P, (R + 1) * H], F32, tag="lgp")
            nc.vector.tensor_tensor(
                out=lgp[:].rearrange("p (k h) -> p k h", h=H),
                in0=alp[:].rearrange("p (k h) -> p k h", h=H),
                in1=arl[:, None, :].to_broadcast([P, R + 1, H]),
                op=ALU.add)
            l2p = lb.tile([P, (R + 1) * H], F32, tag="l2p")
            nc.vector.tensor_scalar_mul(out=l2p[:], in0=lgp[:], scalar1=0.2)
            nc.vector.tensor_tensor(out=lgp[:], in0=lgp[:], in1=l2p[:],
                                    op=ALU.max)
            lm = lb.tile([P, (R + 1) * H], F32, tag="lm")
            nc.vector.tensor_tensor(out=lm[:], in0=lgp[:], in1=maskp[:],
                                    op=ALU.mult)
            mneg = lb.tile([P, (R + 1) * H], F32, tag="mneg")
            nc.vector.tensor_scalar(out=mneg[:], in0=maskp[:], scalar1=1.0,
                                    scalar2=-NEGM, op0=ALU.subtract,
                                    op1=ALU.mult)
            nc.vector.tensor_add(out=lm[:], in0=lm[:], in1=mneg[:])
            ep = lb.tile([P, (R + 1) * H], F32, tag="ep")
            nc.scalar.activation(out=ep[:], in_=lm[:], func=AF.Exp)
            dl = lb.tile([P, H], F32, tag="dl")
            nc.vector.tensor_copy(out=dl[:], in_=ep[:, 0:H])
            for kk in range(1, R + 1):
                nc.vector.tensor_add(out=dl[:], in0=dl[:],
                                     in1=ep[:, kk * H:(kk + 1) * H])
            rl = lb.tile([P, H], F32, tag="rl")
            nc.vector.reciprocal(out=rl[:], in_=dl[:])
            acc = lb.tile([P, D], F32, tag="acc")
            wg = lb.tile([P, H], F32, tag="wg")
            t2 = lb.tile([P, D], F32, tag="t2")
            for kk in range(R + 1):
                nc.vector.tensor_tensor(out=wg[:], in0=ep[:, kk * H:(kk + 1) * H],
                                        in1=rl[:], op=ALU.mult)
                dst_t = acc if kk == 0 else t2
                nc.vector.tensor_tensor(
                    out=dst_t[:].rearrange("p (h c) -> p h c", c=C),
                    in0=vall[kk][:].rearrange("p (h c) -> p h c", c=C),
                    in1=wg[:, :, None].to_broadcast([P, H, C]),
                    op=ALU.mult)
                if kk > 0:
                    nc.vector.tensor_add(out=acc[:], in0=acc[:], in1=t2[:])
            nc.vector.tensor_add(out=acc[:], in0=acc[:], in1=bl_s)
            go = lb.tile([P, D], F32, tag="go")
            nc.scalar.activation(out=go[:], in_=acc[:], func=AF.Gelu)
            # int8-quantize with a per-node scale (host dequantizes); halves
            # the D2H bytes vs f16 at ~0.4%-of-rowmax rounding error
            ab = lb.tile([P, D], F32, tag="ab")
            nc.scalar.activation(out=ab[:], in_=go[:], func=AF.Abs)
            mx = lb.tile([P, 1], F32, tag="mx")
            nc.vector.tensor_reduce(out=mx[:], in_=ab[:], axis=AX.X, op=ALU.max)
            nc.vector.tensor_scalar_max(out=mx[:], in0=mx[:], scalar1=1e-6)
            rq = lb.tile([P, 1], F32, tag="rq")
            nc.vector.reciprocal(out=rq[:], in_=mx[:])
            nc.vector.tensor_scalar_mul(out=rq[:], in0=rq[:], scalar1=127.0)
            qf = lb.tile([P, D], F32, tag="qf")
            nc.vector.tensor_scalar_mul(out=qf[:], in0=go[:], scalar1=rq[:])
            nc.vector.tensor_scalar(out=qf[:], in0=qf[:], scalar1=MAGIC,
                                    scalar2=MAGIC, op0=ALU.add,
                                    op1=ALU.subtract)
            qi = lb.tile([P, D], I8, tag="qi")
            nc.vector.tensor_copy(out=qi[:], in_=qf[:])
            sc = lb.tile([P, 1], F32, tag="sc")
            nc.vector.tensor_scalar_mul(out=sc[:], in0=mx[:], scalar1=1.0 / 127.0)
            nc.gpsimd.dma_start(out=out_q[t * P:(t + 1) * P, 0:D], in_=qi[:])
            nc.gpsimd.dma_start(out=out_q[t * P:(t + 1) * P, D:D + 4],
                                in_=sc[:].bitcast(I8))
    return nc


def _prep(x_inp, edge_index, edge_type, W_self, W_word, att_src_word,
          att_dst_word, bias_word, W_cross, att_src_lang, att_dst_lang,
          bias_lang):
    xpad = np.zeros((NPAD, D), np.float32)
    xpad[:N] = x_inp.astype(np.float32)
    sr = np.maximum(np.abs(xpad).max(axis=1, keepdims=True), 1e-9)
    v = (np.clip(np.rint(xpad * (31.0 / sr)), -31, 31).astype(np.int32) + 32)
    v0, v1, v2, v3 = v[:, 0::4], v[:, 1::4], v[:, 2::4], v[:, 3::4]
    xq = np.concatenate([
        (v0 << 2) | (v1 >> 4),
        ((v1 & 15) << 4) | (v2 >> 2),
        ((v2 & 3) << 6) | v3,
    ], axis=1).astype(np.uint8)                       # [NPAD, 96] planar
    src_all = edge_index[0].astype(np.int64)
    dst_all = edge_index[1].astype(np.int64)
    et_all = edge_type.astype(np.int64)

    # shared params
    Wcat = np.zeros((D, R * FD), np.float32)
    Vcat = np.zeros((D, R * H), np.float32)
    for r in range(R):
        Wr = W_word[r].astype(np.float32)               # [D, D]
        u = np.einsum('dhc,hc->dh', Wr.reshape(D, H, C),
                      att_src_word[r].astype(np.float32))
        v = np.einsum('dhc,hc->dh', Wr.reshape(D, H, C),
                      att_dst_word[r].astype(np.float32))
        Wcat[:, r * FD:r * FD + D] = Wr
        Wcat[:, r * FD + D:(r + 1) * FD] = u
        Vcat[:, r * H:(r + 1) * H] = v
    prow = np.zeros((1, 8 * D), np.float32)
    prow[0, 0:D] = att_src_lang.astype(np.float32).reshape(D)
    prow[0, D:2 * D] = att_dst_lang.astype(np.float32).reshape(D)
    prow[0, 2 * D:3 * D] = bias_lang.astype(np.float32)
    prow[0, 3 * D:8 * D] = bias_word.astype(np.float32).reshape(R * D)
    # device unpacks x in plane-major feature order; permute weight ROWS
    # (x-space) to match; W_cross acts on gelu-space, not x-space
    perm = np.concatenate([np.arange(k, D, 4) for k in range(4)])
    wall = np.concatenate([
        Wcat[perm], Vcat[perm], W_self.astype(np.float32)[perm],
        W_cross.astype(np.float32),
    ], axis=1).astype(ml_dtypes.bfloat16)
    wall_u8 = np.ascontiguousarray(wall).view(np.uint8)        # [P, 2*WC]
    prow_u8 = np.ascontiguousarray(prow.reshape(P, 8)).view(np.uint8)

    # per-core edge binning by (dst tile, relation), fully vectorized:
    # one stable argsort by (core, tile, rel), within-bin rank via cumsum,
    # then a single 2D fancy scatter into the per-core slot tables.
    m_of = dst_all // S
    t_loc = (dst_all - m_of * S) // P
    bin_id = ((m_of * T + t_loc) * R + et_all).astype(np.int32)
    order = np.argsort(bin_id, kind='stable')
    cnts = np.bincount(bin_id, minlength=M * T * R).reshape(M, T, R)
    starts = np.zeros(M * T * R, np.int64)
    starts[1:] = np.cumsum(cnts.reshape(-1))[:-1]
    rank = np.arange(len(order)) - starts[bin_id[order]]

    K = np.maximum(1, -(-cnts.max(axis=0) // P))        # [T, R] chunk counts
    TOTC = int(K.sum())
    coff = np.zeros((T, R), np.int64)                    # chunk offsets
    coff.flat[1:] = np.cumsum(K.flat)[:-1]

    slot = coff.reshape(-1)[(t_loc * R + et_all)[order]] * P + rank
    mo = m_of[order]
    sg = np.zeros((M, TOTC * P), np.uint16)
    du = np.full((M, TOTC * P), 200, np.uint8)
    sg[mo, slot] = src_all[order]
    du[mo, slot] = (dst_all[order] - mo * S) % P

    in_maps = []
    for m in range(M):
        sgT = np.ascontiguousarray(sg[m].reshape(TOTC, P).T)   # [P,TOTC] u16
        duT = np.ascontiguousarray(du[m].reshape(TOTC, P).T)   # [P,TOTC] u8
        xm = np.ascontiguousarray(
            xq[m * S:(m + 1) * S].reshape(T, P, 96)
            .transpose(1, 0, 2).reshape(P, T * 96))
        base = np.concatenate([xm, sgT.view(np.uint8), duT], axis=1)
        pad = np.zeros((P, -(-base.shape[1] // 4) * 4 - base.shape[1]),
                       np.uint8)
        wsh = wall_u8[m * (P // M):(m + 1) * (P // M)].reshape(P, -1)
        et8 = np.concatenate([base, pad, wsh, prow_u8], axis=1)
        in_maps.append({"etab": et8})
    return K.tolist(), TOTC, in_maps


class _CachedExec:
    """Compile the bass program once per program signature and keep the
    jitted SPMD callable; repeat executions then only pay H2D + exec + D2H
    (the intended 'steady-state, compile cached' semantics) instead of
    re-tracing/lowering the ~16k-instruction BIR on every call."""

    def __init__(self, nc):
        import jax
        from jax.sharding import Mesh, PartitionSpec, NamedSharding
        from jax.experimental.shard_map import shard_map
        from concourse import bass2jax
        from concourse.bass2jax import _bass_exec_p, install_neuronx_cc_hook

        install_neuronx_cc_hook()
        self.nc = nc
        in_names, out_names, out_avals, zero_templates = [], [], [], []
        pid = nc.partition_id_tensor.name if nc.partition_id_tensor else None
        for alloc in nc.m.functions[0].allocations:
            if not isinstance(alloc, mybir.MemoryLocationSet):
                continue
            name = alloc.memorylocations[0].name
            if alloc.kind == "ExternalInput":
                if name != pid:
                    in_names.append(name)
            elif alloc.kind == "ExternalOutput":
                out_names.append(name)
                shape = tuple(alloc.tensor_shape)
                dtype = mybir.dt.np(alloc.dtype)
                out_avals.append(jax.core.ShapedArray(shape, dtype))
                zero_templates.append((shape, dtype))
        self.n_params = len(in_names)
        self.in_names = in_names + out_names
        self.out_names = out_names
        if pid is not None:
            self.in_names.append(pid)

        def _body(*args):
            operands = list(args)
            if pid is not None:
                operands.append(bass2jax.partition_id_tensor())
            outs = _bass_exec_p.bind(
                *operands, out_avals=tuple(out_avals),
                in_names=tuple(self.in_names), out_names=tuple(out_names),
                lowering_input_output_aliases=(),
                sim_require_finite=True, sim_require_nnan=True, nc=nc)
            return tuple(outs)

        devices = jax.devices()[:M]
        mesh = Mesh(np.asarray(devices), ("core",))
        n_outs = len(out_names)
        self.sharded = jax.jit(
            shard_map(_body, mesh=mesh,
                      in_specs=(PartitionSpec("core"),) * (self.n_params + n_outs),
                      out_specs=(PartitionSpec("core"),) * n_outs,
                      check_rep=False),
            donate_argnums=tuple(range(self.n_params, self.n_params + n_outs)),
            keep_unused=True)
        # donated output buffers are created ON DEVICE (zeros shipped over
        # the host link every call would be pure transfer waste)
        sh = NamedSharding(mesh, PartitionSpec("core"))
        import jax.numpy as jnp
        self.make_zeros = jax.jit(
            lambda: tuple(jnp.zeros((M * s[0], *s[1:]), d)
                          for s, d in zero_templates),
            out_shardings=tuple(sh for _ in zero_templates))

    def run(self, in_maps):
        # assemble into preallocated pinned-once buffers (reused across
        # calls) instead of np.concatenate's fresh allocation each time
        bufs = getattr(self, "_concat_bufs", None)
        if bufs is None:
            bufs = self._concat_bufs = [
                np.empty((M * in_maps[0][name].shape[0],
                          *in_maps[0][name].shape[1:]),
                         in_maps[0][name].dtype)
                for name in self.in_names[:self.n_params]]
        for i, name in enumerate(self.in_names[:self.n_params]):
            rows = in_maps[0][name].shape[0]
            for c in range(M):
                bufs[i][c * rows:(c + 1) * rows] = in_maps[c][name]
        concat_in = bufs
        # The kernel writes every output element, so the donated output
        # buffers' contents never matter — recycle last call's output arrays
        # instead of materializing fresh device zeros each call.
        donate = getattr(self, "_donate_next", None)
        if donate is None:
            donate = self.make_zeros()
        out_arrs = self.sharded(*concat_in, *donate)
        for o in out_arrs:
            o.copy_to_host_async()
        outs = [np.asarray(o) for o in out_arrs]
        self._donate_next = out_arrs
        return [
            {name: outs[i].reshape(M, -1, *outs[i].shape[1:])[c]
             for i, name in enumerate(self.out_names)}
            for c in range(M)]


_EXEC_CACHE = {}


def _get_exec(K, TOTC):
    key = (tuple(map(tuple, K)), TOTC)
    if key not in _EXEC_CACHE:
        nc = _build(K, TOTC)
        _split_multiwaits(nc)
        _EXEC_CACHE[key] = _CachedExec(nc)
    return _EXEC_CACHE[key]


def rerun():
    """Re-execute the last-compiled program with the last inputs (full
    H2D + device exec + D2H round trip). Used by test.py for steady-state
    timing."""
    return LAST_EXEC.run(LAST_INMAPS)


def kernel(x_inp, node_type, edge_index, edge_type, W_self, W_word,
           att_src_word, att_dst_word, bias_word, W_cross,
           att_src_lang, att_dst_lang, bias_lang):
    global LAST_RESULTS, LAST_NC, LAST_INMAPS, LAST_EXEC
    x_inp = np.asarray(x_inp)
    K, TOTC, in_maps = _prep(
        x_inp, np.asarray(edge_index), np.asarray(edge_type),
        np.asarray(W_self), np.asarray(W_word), np.asarray(att_src_word),
        np.asarray(att_dst_word), np.asarray(bias_word), np.asarray(W_cross),
        np.asarray(att_src_lang), np.asarray(att_dst_lang),
        np.asarray(bias_lang))
    ex = _get_exec(K, TOTC)
    LAST_NC, LAST_INMAPS, LAST_EXEC = ex.nc, in_maps, ex
    results = ex.run(in_maps)
    LAST_RESULTS = None
    buf = np.concatenate([results[m]["out_q"] for m in range(M)], axis=0)[:N]
    q = buf[:, :D].astype(np.float32)
    s = np.ascontiguousarray(buf[:, D:D + 4]).view(np.float32)
    return q * s + x_inp.astype(np.float32)


# revision 28
# speedup vs baseline: 1.0168x; 1.0168x over previous
"""AugGraphConv (per-relation GAT + lang-level softmax) on 8 TRN2 NeuronCores.

v2 — transfer-optimized (the axon tunnel at ~35-80MB/s dominates wall):
  - x is SHARDED: core m receives only rows [m*S,(m+1)*S), quantized to
    per-row int8 (LayerNorm is scale-invariant per row, so no scales ship).
    Stage A (LN + per-relation features) runs on owned rows only; the bf16
    feature tables are AllGathered on-device (5 x 13.7MB over NeuronLink,
    Shared output buffers).
  - ONE u8 input array per core carries [x int8 | src-id u16 bytes |
    dst-slot u8]; gather offsets (i32) / one-hot keys (bf16) are
    upconverted on device. Weights ride in one bf16 array; attention/bias
    rows ship as a [1,1024] f32 row broadcast on device with a K=1 matmul;
    iota/identity are NEFF consts.
  - Output is gelu(...) WITHOUT the +x residual (host adds x_inp in f32),
    int8-quantized with a per-node f32 scale bitcast into the same array.
  - The compiled SPMD executable is CACHED (_CachedExec); repeat calls pay
    only H2D + exec + D2H, not jit re-trace/lowering of the ~15k-inst BIR.
    Donated output buffers are recycled from the previous call.
Compute structure (per core, dst-sharded graph parallel):
  edges binned by (own dst tile, relation) into 128-slot chunks; segment
  softmax without max-subtraction; one-hot scatter-add via PE matmuls.
"""

import os
import numpy as np
import ml_dtypes
from contextlib import ExitStack

import concourse.bass as bass
import concourse.mybir as mybir
from concourse.bass import IndirectOffsetOnAxis
from concourse.tile import TileContext
from concourse.bass_utils import run_bass_kernel_spmd

N, D, H, R, C = 50000, 128, 8, 5, 16
P = 128
M = 8
NPAD = 50176            # 392 * 128, divisible by M*P
S = NPAD // M           # 6272 rows per core
T = S // P              # 49 owned tiles per core
FD = D + H              # 136: [xw | al]
ARPAD = R * S + 2 * P   # arrel rows incl. pad region (covers idx <= 31432)
F32 = mybir.dt.float32
F16 = mybir.dt.float16
BF16 = mybir.dt.bfloat16
I32 = mybir.dt.int32
I8 = mybir.dt.int8
U16 = mybir.dt.uint16
U8 = mybir.dt.uint8
MAGIC = 12582912.0      # 1.5*2^23: (v+MAGIC)-MAGIC rounds f32 to nearest int
AF = mybir.ActivationFunctionType
ALU = mybir.AluOpType
AX = mybir.AxisListType
NEGM = -30.0            # softmax mask value (exp(-30) ~ 1e-13, negligible)

LAST_RESULTS = None     # test.py reads exec_time_ns / profile from here


def _split_multiwaits(nc):
    """This toolchain's walrus codegen allows only one sem-wait per
    instruction; hoist extra waits into preceding NoOps on the same engine
    (sequencer executes them in program order, so semantics are identical)."""
    n_split = 0
    for _, bbwrap in nc.bb_map.items():
        bb = bbwrap.bb
        out = []
        changed = False
        for inst in list(bb.instructions):
            si = inst.sync_info
            if si is not None and si.on_wait is not None and len(si.on_wait) > 1:
                waits = list(si.on_wait)
                for w in waits[:-1]:
                    out.append(mybir.InstNoOp(
                        name=nc.get_next_instruction_name(),
                        engine=inst.engine, ins=[], outs=[],
                        sync_info=mybir.SyncInfo(on_wait=[w], on_update=[])))
                    n_split += 1
                si.on_wait = waits[-1:]
                inst.sync_info = si
                changed = True
            out.append(inst)
        if changed:
            bb.instructions = out
    return n_split


def _build(K, TOTC):
    nc = bass.Bass(num_devices=M)
    # merged I/O (the axon tunnel has substantial per-array overhead):
    #   wall = [wcat(680) | vcat(40) | wself(128) | wcross(128)] bf16
    #   etab = [x int8 tile-blocks (T*D) | src u16 bytes (2*TOTC) |
    #           dst u8 (TOTC)] per partition
    #   out_q = [int8 q (D) | f32 row-scale bitcast (4)] per row
    XW = 96                                     # packed x bytes per tile row
    TD = T * XW
    WC = R * FD + R * H + 2 * D                 # wall bf16 columns (976)
    WSB = 2 * WC // M                           # wall-shard bytes/partition
    WOFF = -(-(TD + 3 * TOTC) // 4) * 4         # 4B-aligned shard byte offset
    POFF = WOFF + WSB                           # prow f32 byte offset
    ECOLS = POFF + 32
    etab = nc.declare_dram_parameter("etab", [P, ECOLS], U8, isOutput=False)
    out_q = nc.declare_dram_parameter("out_q", [S, D + 4], I8, isOutput=True)

    ident_d = nc.inline_tensor(np.eye(P, dtype=np.float32), name="ident_c")
    iorow_d = nc.inline_tensor(
        np.tile(np.arange(P, dtype=np.float32)[None, :], (P, 1))
        .astype(ml_dtypes.bfloat16), name="iorow_c")

    wl_loc = nc.dram_tensor("wl_loc", [P // M, 2 * WC], U8)
    wl_glob = nc.dram_tensor("wl_glob", [P, 2 * WC], U8, addr_space="Shared")
    featl = [nc.dram_tensor(f"featl{r}", [S, FD], BF16) for r in range(R)]
    featg = [nc.dram_tensor(f"featg{r}", [NPAD, FD], BF16, addr_space="Shared")
             for r in range(R)]
    arrel = nc.dram_tensor("ar_rel", [ARPAD, H], BF16)

    groups = [list(range(M))]

    with TileContext(nc) as tc, ExitStack() as ctx:
        cp = ctx.enter_context(tc.tile_pool(name="const", bufs=1))
        so = ctx.enter_context(tc.tile_pool(name="sown", bufs=1))
        sb = ctx.enter_context(tc.tile_pool(name="sb", bufs=3))
        eb = ctx.enter_context(tc.tile_pool(name="eb", bufs=4))
        lb = ctx.enter_context(tc.tile_pool(name="lb", bufs=2))
        psA = ctx.enter_context(tc.tile_pool(name="psA", bufs=2, space="PSUM"))
        psB = ctx.enter_context(tc.tile_pool(name="psB", bufs=1, space="PSUM"))

        # ---- persistent constants / index arrays ----
        # wall weights are SHARDED across cores inside etab: bounce this
        # core's [P, WSB] slice to DRAM as [P/M, 2*WC] (same flat order),
        # AllGather to the full [P, 2*WC], then re-type to a bf16 tile.
        wsh_s = cp.tile([P, WSB], U8)
        nc.gpsimd.dma_start(out=wsh_s[:], in_=etab[:, WOFF:WOFF + WSB])
        nc.gpsimd.dma_start(out=wl_loc[:], in_=wsh_s[:])
        nc.gpsimd.collective_compute(
            "AllGather", ALU.bypass, replica_groups=groups,
            ins=[wl_loc[:]], outs=[wl_glob[:]])
        wall_s = cp.tile([D, WC], BF16)
        nc.gpsimd.dma_start(out=wall_s[:], in_=wl_glob[:].bitcast(BF16))
        vcat_s = wall_s[:, R * FD:R * FD + R * H]
        wself_s = wall_s[:, R * FD + R * H:R * FD + R * H + D]
        wcross_s = wall_s[:, R * FD + R * H + D:R * FD + R * H + 2 * D]
        iden_s = cp.tile([P, P], F32)
        nc.gpsimd.dma_start(out=iden_s[:], in_=ident_d[:])
        iorow_s = cp.tile([P, P], BF16)
        nc.gpsimd.dma_start(out=iorow_s[:], in_=iorow_d[:])

        etab_s = cp.tile([P, TD + 3 * TOTC], U8)
        nc.gpsimd.dma_start(out=etab_s[:], in_=etab[:, 0:TD + 3 * TOTC])
        srcu_s = etab_s[:, TD:TD + 2 * TOTC].bitcast(U16)  # [P,TOTC] u16 view
        dstu_s = etab_s[:, TD + 2 * TOTC:TD + 3 * TOTC]
        srci_s = cp.tile([P, TOTC], I32)
        nc.vector.tensor_copy(out=srci_s[:], in_=srcu_s)
        dsti_s = cp.tile([P, TOTC], I32)
        nc.vector.tensor_copy(out=dsti_s[:], in_=dstu_s)
        dstb_s = cp.tile([P, TOTC], BF16)
        nc.vector.tensor_copy(out=dstb_s[:], in_=dstu_s)

        # ---- broadcast param row [1,1024] -> [P,1024] via K=1 matmul ----
        # [P,8] f32 region linearizes row-major to the [1,1024] prow row
        prow_s = cp.tile([1, 8 * D], F32)
        nc.gpsimd.dma_start(out=prow_s[:],
                            in_=etab[:, POFF:POFF + 32].bitcast(F32))
        ones_s = cp.tile([1, P], F32)
        nc.vector.memset(ones_s[:], 1.0)
        params_s = cp.tile([P, 8 * D], F32)
        for h in range(8):
            pr_ps = psA.tile([P, P], F32, tag="tp")
            nc.tensor.matmul(out=pr_ps[:], lhsT=ones_s[:],
                             rhs=prow_s[:, h * D:(h + 1) * D],
                             start=True, stop=True)
            nc.vector.tensor_copy(out=params_s[:, h * D:(h + 1) * D],
                                  in_=pr_ps[:])
        asl_s = params_s[:, 0:D]
        adl_s = params_s[:, D:2 * D]
        bl_s = params_s[:, 2 * D:3 * D]
        bw_s = params_s[:, 3 * D:8 * D]

        # zero the arrel pad region (gathers may touch rows >= R*S)
        zero_s = cp.tile([P, H], BF16)
        nc.vector.memset(zero_s[:], 0.0)
        for i in range(2):
            nc.gpsimd.dma_start(out=arrel[R * S + i * P:R * S + (i + 1) * P, :],
                                in_=zero_s[:])

        # ---- Stage A: LN + per-relation features for OWNED nodes only ----
        sown_tiles = []
        for t in range(T):
            # unpack 4x6-bit planar x: section [b0(32)|b1(32)|b2(32)] bytes;
            # feature order is plane-major (host permuted weight rows to match)
            xo = t * XW
            bi = sb.tile([P, XW], I32, tag="bi")
            nc.vector.tensor_copy(out=bi[:], in_=etab_s[:, xo:xo + XW])
            i0, i1, i2 = bi[:, 0:32], bi[:, 32:64], bi[:, 64:96]
            xi = sb.tile([P, D], I32, tag="xi")
            ta = sb.tile([P, 32], I32, tag="ta")
            tb = sb.tile([P, 32], I32, tag="tb")
            nc.vector.tensor_scalar(out=xi[:, 0:32], in0=i0, scalar1=2,
                                    scalar2=None, op0=ALU.logical_shift_right)
            nc.vector.tensor_scalar(out=ta[:], in0=i0, scalar1=3, scalar2=None,
                                    op0=ALU.bitwise_and)
            nc.vector.tensor_scalar(out=ta[:], in0=ta[:], scalar1=4,
                                    scalar2=None, op0=ALU.logical_shift_left)
            nc.vector.tensor_scalar(out=tb[:], in0=i1, scalar1=4, scalar2=None,
                                    op0=ALU.logical_shift_right)
            nc.vector.tensor_tensor(out=xi[:, 32:64], in0=ta[:], in1=tb[:],
                                    op=ALU.bitwise_or)
            nc.vector.tensor_scalar(out=ta[:], in0=i1, scalar1=15, scalar2=None,
                                    op0=ALU.bitwise_and)
            nc.vector.tensor_scalar(out=ta[:], in0=ta[:], scalar1=2,
                                    scalar2=None, op0=ALU.logical_shift_left)
            nc.vector.tensor_scalar(out=tb[:], in0=i2, scalar1=6, scalar2=None,
                                    op0=ALU.logical_shift_right)
            nc.vector.tensor_tensor(out=xi[:, 64:96], in0=ta[:], in1=tb[:],
                                    op=ALU.bitwise_or)
            nc.vector.tensor_scalar(out=xi[:, 96:128], in0=i2, scalar1=63,
                                    scalar2=None, op0=ALU.bitwise_and)
            xt = sb.tile([P, D], F32, tag="xt")
            nc.vector.tensor_scalar(out=xt[:], in0=xi[:], scalar1=32,
                                    scalar2=None, op0=ALU.subtract)
            mu = sb.tile([P, 1], F32, tag="mu")
            nc.vector.tensor_reduce(out=mu[:], in_=xt[:], axis=AX.X, op=ALU.add)
            nc.vector.tensor_scalar_mul(out=mu[:], in0=mu[:], scalar1=1.0 / D)
            xc = sb.tile([P, D], F32, tag="xc")
            nc.vector.tensor_scalar(out=xc[:], in0=xt[:], scalar1=mu[:],
                                    scalar2=None, op0=ALU.subtract)
            sq = sb.tile([P, D], F32, tag="sq")
            nc.scalar.activation(out=sq[:], in_=xc[:], func=AF.Square)
            var = sb.tile([P, 1], F32, tag="var")
            nc.vector.tensor_reduce(out=var[:], in_=sq[:], axis=AX.X, op=ALU.add)
            nc.vector.tensor_scalar(out=var[:], in0=var[:], scalar1=1.0 / D,
                                    scalar2=1e-5, op0=ALU.mult, op1=ALU.add)
            sd = sb.tile([P, 1], F32, tag="sd")
            nc.scalar.activation(out=sd[:], in_=var[:], func=AF.Sqrt)
            rs = sb.tile([P, 1], F32, tag="rs")
            nc.vector.reciprocal(out=rs[:], in_=sd[:])
            xn = sb.tile([P, D], F32, tag="xn")
            nc.vector.tensor_scalar_mul(out=xn[:], in0=xc[:], scalar1=rs[:])
            tp = psA.tile([P, P], F32, tag="tp")
            nc.tensor.transpose(out=tp[:], in_=xn[:], identity=iden_s[:])
            xnT = sb.tile([P, P], BF16, tag="xnT")
            nc.vector.tensor_copy(out=xnT[:], in_=tp[:])
            for r in range(R):
                fm = psA.tile([P, FD], F32, tag="fm")
                nc.tensor.matmul(out=fm[:], lhsT=xnT[:],
                                 rhs=wall_s[:, r * FD:(r + 1) * FD],
                                 start=True, stop=True)
                fc = sb.tile([P, FD], BF16, tag="fc")
                nc.vector.tensor_copy(out=fc[:], in_=fm[:])
                nc.gpsimd.dma_start(out=featl[r][t * P:(t + 1) * P, :], in_=fc[:])
            am = psA.tile([P, FD], F32, tag="fm")
            nc.tensor.matmul(out=am[:, :R * H], lhsT=xnT[:], rhs=vcat_s,
                             start=True, stop=True)
            ac = sb.tile([P, R * H], BF16, tag="ac")
            nc.vector.tensor_copy(out=ac[:], in_=am[:, :R * H])
            for r in range(R):
                nc.gpsimd.dma_start(
                    out=arrel[r * S + t * P:r * S + (t + 1) * P, :],
                    in_=ac[:, r * H:(r + 1) * H])
            sm_ = psA.tile([P, FD], F32, tag="fm")
            nc.tensor.matmul(out=sm_[:, :D], lhsT=xnT[:], rhs=wself_s,
                             start=True, stop=True)
            sc = so.tile([P, D], F32, tag=f"sown{t}")
            nc.vector.tensor_copy(out=sc[:], in_=sm_[:, :D])
            sown_tiles.append(sc)

        # ---- AllGather per-relation feature tables across the 8 cores ----
        for r in range(R):
            nc.gpsimd.collective_compute(
                "AllGather", ALU.bypass, replica_groups=groups,
                ins=[featl[r][:]], outs=[featg[r][:]])

        # ---- Stage B: edge aggregation + lang softmax, per owned tile ----
        c = 0
        for t in range(T):
            maskp = lb.tile([P, (R + 1) * H], F32, tag="maskp")
            nc.vector.memset(maskp[:, 0:H], 1.0)
            vts = []
            for r in range(R):
                Kt = K[t][r]
                nd_ps = psB.tile([P, FD], F32, tag="nd")
                num_ps = nd_ps[:, 0:D]
                den_ps = nd_ps[:, D:FD]
                for k in range(Kt):
                    G = eb.tile([P, FD], BF16, tag="G")
                    nc.gpsimd.indirect_dma_start(
                        out=G[:], out_offset=None, in_=featg[r][:],
                        in_offset=IndirectOffsetOnAxis(ap=srci_s[:, c:c + 1], axis=0))
                    ari = eb.tile([P, 1], I32, tag="ari")
                    nc.vector.tensor_scalar(out=ari[:], in0=dsti_s[:, c:c + 1],
                                            scalar1=r * S + t * P, scalar2=None,
                                            op0=ALU.add)
                    Aar = eb.tile([P, H], BF16, tag="Aar")
                    nc.gpsimd.indirect_dma_start(
                        out=Aar[:], out_offset=None, in_=arrel[:],
                        in_offset=IndirectOffsetOnAxis(ap=ari[:], axis=0))
                    lg = eb.tile([P, H], F32, tag="lg")
                    nc.vector.tensor_add(out=lg[:], in0=G[:, D:FD], in1=Aar[:])
                    l2 = eb.tile([P, H], F32, tag="l2")
                    nc.vector.tensor_scalar_mul(out=l2[:], in0=lg[:], scalar1=0.2)
                    lr = eb.tile([P, H], F32, tag="lr")
                    nc.vector.tensor_tensor(out=lr[:], in0=lg[:], in1=l2[:],
                                            op=ALU.max)
                    wb = eb.tile([P, H], BF16, tag="wb")
                    nc.scalar.activation(out=wb[:], in_=lr[:], func=AF.Exp)
                    V = eb.tile([P, FD], BF16, tag="V")
                    nc.vector.tensor_copy(out=V[:, D:FD], in_=wb[:])
                    Sm = eb.tile([P, P], BF16, tag="Sm")
                    nc.vector.tensor_tensor(
                        out=Sm[:], in0=dstb_s[:, c:c + 1].to_broadcast([P, P]),
                        in1=iorow_s[:], op=ALU.is_equal)
                    nc.vector.tensor_tensor(
                        out=V[:, 0:D].rearrange("p (h c) -> p h c", c=C),
                        in0=G[:, 0:D].rearrange("p (h c) -> p h c", c=C),
                        in1=wb[:, :, None].to_broadcast([P, H, C]),
                        op=ALU.mult)
                    nc.tensor.matmul(out=nd_ps[:], lhsT=Sm[:], rhs=V[:],
                                     start=(k == 0), stop=(k == Kt - 1))
                    c += 1
                den1 = eb.tile([P, H], F32, tag="den1")
                nc.vector.tensor_scalar_max(out=den1[:], in0=den_ps[:],
                                            scalar1=1e-6)
                rec = eb.tile([P, H], F32, tag="rec")
                nc.vector.reciprocal(out=rec[:], in_=den1[:])
                nc.vector.tensor_scalar(
                    out=maskp[:, (r + 1) * H:(r + 2) * H], in0=den_ps[:],
                    scalar1=0.0, scalar2=None, op0=ALU.is_gt)
                O = eb.tile([P, D], F32, tag="O")
                nc.vector.tensor_tensor(
                    out=O[:].rearrange("p (h c) -> p h c", c=C),
                    in0=num_ps[:].rearrange("p (h c) -> p h c", c=C),
                    in1=rec[:, :, None].to_broadcast([P, H, C]),
                    op=ALU.mult)
                nc.vector.tensor_add(out=O[:], in0=O[:],
                                     in1=bw_s[:, r * D:(r + 1) * D])
                g = eb.tile([P, D], F32, tag="g")
                nc.scalar.activation(out=g[:], in_=O[:], func=AF.Gelu)
                tpb = psA.tile([P, P], F32, tag="tp")
                nc.tensor.transpose(out=tpb[:], in_=g[:], identity=iden_s[:])
                gT = eb.tile([P, P], BF16, tag="gT")
                nc.vector.tensor_copy(out=gT[:], in_=tpb[:])
                v_ps = psB.tile([P, D], F32, tag="vps")
                nc.tensor.matmul(out=v_ps[:], lhsT=gT[:], rhs=wcross_s,
                                 start=True, stop=True)
                vr = lb.tile([P, D], F32, tag=f"v{r + 1}")
                nc.vector.tensor_copy(out=vr[:], in_=v_ps[:])
                vts.append(vr)

            # lang-level GAT over 6 feature rows for this tile
            v0 = sown_tiles[t]
            vall = [v0] + vts
            alp = lb.tile([P, (R + 1) * H], F32, tag="alp")
            tmp = lb.tile([P, D], F32, tag="ltmp")
            for kk in range(R + 1):
                nc.vector.tensor_tensor(out=tmp[:], in0=vall[kk][:],
                                        in1=asl_s, op=ALU.mult)
                nc.vector.tensor_reduce(
                    out=alp[:, kk * H:(kk + 1) * H],
                    in_=tmp[:].rearrange("p (h c) -> p h c", c=C),
                    axis=AX.X, op=ALU.add)
            arl = lb.tile([P, H], F32, tag="arl")
            nc.vector.tensor_tensor(out=tmp[:], in0=v0[:], in1=adl_s,
                                    op=ALU.mult)
            nc.vector.tensor_reduce(
                out=arl[:], in_=tmp[:].rearrange("p (h c) -> p h c", c=C),
                axis=AX.X, op=ALU.add)
            lgp = lb.tile([P, (R + 1) * H], F32, tag="lgp")
            nc.vector.tensor_tensor(
                out=lgp[:].rearrange("p (k h) -> p k h", h=H),
                in0=alp[:].rearrange("p (k h) -> p k h", h=H),
                in1=arl[:, None, :].to_broadcast([P, R + 1, H]),
                op=ALU.add)
            l2p = lb.tile([P, (R + 1) * H], F32, tag="l2p")
            nc.vector.tensor_scalar_mul(out=l2p[:], in0=lgp[:], scalar1=0.2)
            nc.vector.tensor_tensor(out=lgp[:], in0=lgp[:], in1=l2p[:],
                                    op=ALU.max)
            lm = lb.tile([P, (R + 1) * H], F32, tag="lm")
            nc.vector.tensor_tensor(out=lm[:], in0=lgp[:], in1=maskp[:],
                                    op=ALU.mult)
            mneg = lb.tile([P, (R + 1) * H], F32, tag="mneg")
            nc.vector.tensor_scalar(out=mneg[:], in0=maskp[:], scalar1=1.0,
                                    scalar2=-NEGM, op0=ALU.subtract,
                                    op1=ALU.mult)
            nc.vector.tensor_add(out=lm[:], in0=lm[:], in1=mneg[:])
            ep = lb.tile([P, (R + 1) * H], F32, tag="ep")
            nc.scalar.activation(out=ep[:], in_=lm[:], func=AF.Exp)
            dl = lb.tile([P, H], F32, tag="dl")
            nc.vector.tensor_copy(out=dl[:], in_=ep[:, 0:H])
            for kk in range(1, R + 1):
                nc.vector.tensor_add(out=dl[:], in0=dl[:],
                                     in1=ep[:, kk * H:(kk + 1) * H])
            rl = lb.tile([P, H], F32, tag="rl")
            nc.vector.reciprocal(out=rl[:], in_=dl[:])
            acc = lb.tile([P, D], F32, tag="acc")
            wg = lb.tile([P, H], F32, tag="wg")
            t2 = lb.tile([P, D], F32, tag="t2")
            for kk in range(R + 1):
                nc.vector.tensor_tensor(out=wg[:], in0=ep[:, kk * H:(kk + 1) * H],
                                        in1=rl[:], op=ALU.mult)
                dst_t = acc if kk == 0 else t2
                nc.vector.tensor_tensor(
                    out=dst_t[:].rearrange("p (h c) -> p h c", c=C),
                    in0=vall[kk][:].rearrange("p (h c) -> p h c", c=C),
                    in1=wg[:, :, None].to_broadcast([P, H, C]),
                    op=ALU.mult)
                if kk > 0:
                    nc.vector.tensor_add(out=acc[:], in0=acc[:], in1=t2[:])
            nc.vector.tensor_add(out=acc[:], in0=acc[:], in1=bl_s)
            go = lb.tile([P, D], F32, tag="go")
            nc.scalar.activation(out=go[:], in_=acc[:], func=AF.Gelu)
            # int8-quantize with a per-node scale (host dequantizes); halves
            # the D2H bytes vs f16 at ~0.4%-of-rowmax rounding error
            ab = lb.tile([P, D], F32, tag="ab")
            nc.scalar.activation(out=ab[:], in_=go[:], func=AF.Abs)
            mx = lb.tile([P, 1], F32, tag="mx")
            nc.vector.tensor_reduce(out=mx[:], in_=ab[:], axis=AX.X, op=ALU.max)
            nc.vector.tensor_scalar_max(out=mx[:], in0=mx[:], scalar1=1e-6)
            rq = lb.tile([P, 1], F32, tag="rq")
            nc.vector.reciprocal(out=rq[:], in_=mx[:])
            nc.vector.tensor_scalar_mul(out=rq[:], in0=rq[:], scalar1=127.0)
            qf = lb.tile([P, D], F32, tag="qf")
            nc.vector.tensor_scalar_mul(out=qf[:], in0=go[:], scalar1=rq[:])
            nc.vector.tensor_scalar(out=qf[:], in0=qf[:], scalar1=MAGIC,
                                    scalar2=MAGIC, op0=ALU.add,
                                    op1=ALU.subtract)
            qi = lb.tile([P, D], I8, tag="qi")
            nc.vector.tensor_copy(out=qi[:], in_=qf[:])
            sc = lb.tile([P, 1], F32, tag="sc")
            nc.vector.tensor_scalar_mul(out=sc[:], in0=mx[:], scalar1=1.0 / 127.0)
            nc.gpsimd.dma_start(out=out_q[t * P:(t + 1) * P, 0:D], in_=qi[:])
            nc.gpsimd.dma_start(out=out_q[t * P:(t + 1) * P, D:D + 4],
                                in_=sc[:].bitcast(I8))
    return nc


def _prep(x_inp, edge_index, edge_type, W_self, W_word, att_src_word,
          att_dst_word, bias_word, W_cross, att_src_lang, att_dst_lang,
          bias_lang):
    xpad = np.zeros((NPAD, D), np.float32)
    xpad[:N] = x_inp.astype(np.float32)
    sr = np.maximum(np.abs(xpad).max(axis=1, keepdims=True), 1e-9)
    v = (np.clip(np.rint(xpad * (31.0 / sr)), -31, 31).astype(np.int32) + 32)
    v0, v1, v2, v3 = v[:, 0::4], v[:, 1::4], v[:, 2::4], v[:, 3::4]
    xq = np.concatenate([
        (v0 << 2) | (v1 >> 4),
        ((v1 & 15) << 4) | (v2 >> 2),
        ((v2 & 3) << 6) | v3,
    ], axis=1).astype(np.uint8)                       # [NPAD, 96] planar
    src_all = edge_index[0].astype(np.int64)
    dst_all = edge_index[1].astype(np.int64)
    et_all = edge_type.astype(np.int64)

    # shared params
    Wcat = np.zeros((D, R * FD), np.float32)
    Vcat = np.zeros((D, R * H), np.float32)
    for r in range(R):
        Wr = W_word[r].astype(np.float32)               # [D, D]
        u = np.einsum('dhc,hc->dh', Wr.reshape(D, H, C),
                      att_src_word[r].astype(np.float32))
        v = np.einsum('dhc,hc->dh', Wr.reshape(D, H, C),
                      att_dst_word[r].astype(np.float32))
        Wcat[:, r * FD:r * FD + D] = Wr
        Wcat[:, r * FD + D:(r + 1) * FD] = u
        Vcat[:, r * H:(r + 1) * H] = v
    prow = np.zeros((1, 8 * D), np.float32)
    prow[0, 0:D] = att_src_lang.astype(np.float32).reshape(D)
    prow[0, D:2 * D] = att_dst_lang.astype(np.float32).reshape(D)
    prow[0, 2 * D:3 * D] = bias_lang.astype(np.float32)
    prow[0, 3 * D:8 * D] = bias_word.astype(np.float32).reshape(R * D)
    # device unpacks x in plane-major feature order; permute weight ROWS
    # (x-space) to match; W_cross acts on gelu-space, not x-space
    perm = np.concatenate([np.arange(k, D, 4) for k in range(4)])
    wall = np.concatenate([
        Wcat[perm], Vcat[perm], W_self.astype(np.float32)[perm],
        W_cross.astype(np.float32),
    ], axis=1).astype(ml_dtypes.bfloat16)
    wall_u8 = np.ascontiguousarray(wall).view(np.uint8)        # [P, 2*WC]
    prow_u8 = np.ascontiguousarray(prow.reshape(P, 8)).view(np.uint8)

    # per-core edge binning by (dst tile, relation), fully vectorized:
    # one stable argsort by (core, tile, rel), within-bin rank via cumsum,
    # then a single 2D fancy scatter into the per-core slot tables.
    m_of = dst_all // S
    t_loc = (dst_all - m_of * S) // P
    bin_id = ((m_of * T + t_loc) * R + et_all).astype(np.int32)
    order = np.argsort(bin_id, kind='stable')
    cnts = np.bincount(bin_id, minlength=M * T * R).reshape(M, T, R)
    starts = np.zeros(M * T * R, np.int64)
    starts[1:] = np.cumsum(cnts.reshape(-1))[:-1]
    rank = np.arange(len(order)) - starts[bin_id[order]]

    K = np.maximum(1, -(-cnts.max(axis=0) // P))        # [T, R] chunk counts
    TOTC = int(K.sum())
    coff = np.zeros((T, R), np.int64)                    # chunk offsets
    coff.flat[1:] = np.cumsum(K.flat)[:-1]

    slot = coff.reshape(-1)[(t_loc * R + et_all)[order]] * P + rank
    mo = m_of[order]
    sg = np.zeros((M, TOTC * P), np.uint16)
    du = np.full((M, TOTC * P), 200, np.uint8)
    sg[mo, slot] = src_all[order]
    du[mo, slot] = (dst_all[order] - mo * S) % P

    in_maps = []
    for m in range(M):
        sgT = np.ascontiguousarray(sg[m].reshape(TOTC, P).T)   # [P,TOTC] u16
        duT = np.ascontiguousarray(du[m].reshape(TOTC, P).T)   # [P,TOTC] u8
        xm = np.ascontiguousarray(
            xq[m * S:(m + 1) * S].reshape(T, P, 96)
            .transpose(1, 0, 2).reshape(P, T * 96))
        base = np.concatenate([xm, sgT.view(np.uint8), duT], axis=1)
        pad = np.zeros((P, -(-base.shape[1] // 4) * 4 - base.shape[1]),
                       np.uint8)
        wsh = wall_u8[m * (P // M):(m + 1) * (P // M)].reshape(P, -1)
        et8 = np.concatenate([base, pad, wsh, prow_u8], axis=1)
        in_maps.append({"etab": et8})
    return K.tolist(), TOTC, in_maps


class _CachedExec:
    """Compile the bass program once per program signature and keep the
    jitted SPMD callable; repeat executions then only pay H2D + exec + D2H
    (the intended 'steady-state, compile cached' semantics) instead of
    re-tracing/lowering the ~16k-instruction BIR on every call."""

    def __init__(self, nc):
        import jax
        from jax.sharding import Mesh, PartitionSpec, NamedSharding
        from jax.experimental.shard_map import shard_map
        from concourse import bass2jax
        from concourse.bass2jax import _bass_exec_p, install_neuronx_cc_hook

        install_neuronx_cc_hook()
        self.nc = nc
        in_names, out_names, out_avals, zero_templates = [], [], [], []
        pid = nc.partition_id_tensor.name if nc.partition_id_tensor else None
        for alloc in nc.m.functions[0].allocations:
            if not isinstance(alloc, mybir.MemoryLocationSet):
                continue
            name = alloc.memorylocations[0].name
            if alloc.kind == "ExternalInput":
                if name != pid:
                    in_names.append(name)
            elif alloc.kind == "ExternalOutput":
                out_names.append(name)
                shape = tuple(alloc.tensor_shape)
                dtype = mybir.dt.np(alloc.dtype)
                out_avals.append(jax.core.ShapedArray(shape, dtype))
                zero_templates.append((shape, dtype))
        self.n_params = len(in_names)
        self.in_names = in_names + out_names
        self.out_names = out_names
        if pid is not None:
            self.in_names.append(pid)

        def _body(*args):
            operands = list(args)
            if pid is not None:
                operands.append(bass2jax.partition_id_tensor())
            outs = _bass_exec_p.bind(
                *operands, out_avals=tuple(out_avals),
                in_names=tuple(self.in_names), out_names=tuple(out_names),
                lowering_input_output_aliases=(),
                sim_require_finite=True, sim_require_nnan=True, nc=nc)
            return tuple(outs)

        devices = jax.devices()[:M]
        mesh = Mesh(np.asarray(devices), ("core",))
        n_outs = len(out_names)
        self.sharded = jax.jit(
            shard_map(_body, mesh=mesh,
                      in_specs=(PartitionSpec("core"),) * (self.n_params + n_outs),
                      out_specs=(PartitionSpec("core"),) * n_outs,
                      check_rep=False),
            donate_argnums=tuple(range(self.n_params, self.n_params + n_outs)),
            keep_unused=True)
        # donated output buffers are created ON DEVICE (zeros shipped over
        # the host link every call would be pure transfer waste)
        sh = NamedSharding(mesh, PartitionSpec("core"))
        import jax.numpy as jnp
        self.make_zeros = jax.jit(
            lambda: tuple(jnp.zeros((M * s[0], *s[1:]), d)
                          for s, d in zero_templates),
            out_shardings=tuple(sh for _ in zero_templates))

    def run(self, in_maps):
        # assemble into preallocated pinned-once buffers (reused across
        # calls) instead of np.concatenate's fresh allocation each time
        bufs = getattr(self, "_concat_bufs", None)
        if bufs is None:
            bufs = self._concat_bufs = [
                np.empty((M * in_maps[0][name].shape[0],
                          *in_maps[0][name].shape[1:]),
                         in_maps[0][name].dtype)
                for name in self.in_names[:self.n_params]]
        for i, name in enumerate(self.in_names[:self.n_params]):
            rows = in_maps[0][name].shape[0]
            for c in range(M):
                bufs[i][c * rows:(c + 1) * rows] = in_maps[c][name]
        concat_in = bufs
        # The kernel writes every output element, so the donated output
        # buffers' contents never matter — recycle last call's output arrays
        # instead of materializing fresh device zeros each call.
        donate = getattr(self, "_donate_next", None)
        if donate is None:
            donate = self.make_zeros()
        out_arrs = self.sharded(*concat_in, *donate)
        # fetch via an explicit transfer to the CPU backend when available
        # (slightly faster than the blocking np.asarray path on this tunnel)
        try:
            import jax
            cpu0 = getattr(self, "_cpu0", None)
            if cpu0 is None:
                cpu0 = self._cpu0 = jax.devices("cpu")[0]
            outs = [np.asarray(jax.device_put(o, cpu0)) for o in out_arrs]
        except Exception:
            for o in out_arrs:
                o.copy_to_host_async()
            outs = [np.asarray(o) for o in out_arrs]
        self._donate_next = out_arrs
        return [
            {name: outs[i].reshape(M, -1, *outs[i].shape[1:])[c]
             for i, name in enumerate(self.out_names)}
            for c in range(M)]


_EXEC_CACHE = {}


def _get_exec(K, TOTC):
    key = (tuple(map(tuple, K)), TOTC)
    if key not in _EXEC_CACHE:
        nc = _build(K, TOTC)
        _split_multiwaits(nc)
        _EXEC_CACHE[key] = _CachedExec(nc)
    return _EXEC_CACHE[key]


def rerun():
    """Re-execute the last-compiled program with the last inputs (full
    H2D + device exec + D2H round trip). Used by test.py for steady-state
    timing."""
    return LAST_EXEC.run(LAST_INMAPS)


def kernel(x_inp, node_type, edge_index, edge_type, W_self, W_word,
           att_src_word, att_dst_word, bias_word, W_cross,
           att_src_lang, att_dst_lang, bias_lang):
    global LAST_RESULTS, LAST_NC, LAST_INMAPS, LAST_EXEC
    x_inp = np.asarray(x_inp)
    K, TOTC, in_maps = _prep(
        x_inp, np.asarray(edge_index), np.asarray(edge_type),
        np.asarray(W_self), np.asarray(W_word), np.asarray(att_src_word),
        np.asarray(att_dst_word), np.asarray(bias_word), np.asarray(W_cross),
        np.asarray(att_src_lang), np.asarray(att_dst_lang),
        np.asarray(bias_lang))
    ex = _get_exec(K, TOTC)
    LAST_NC, LAST_INMAPS, LAST_EXEC = ex.nc, in_maps, ex
    results = ex.run(in_maps)
    LAST_RESULTS = None
    buf = np.concatenate([results[m]["out_q"] for m in range(M)], axis=0)[:N]
    q = buf[:, :D].astype(np.float32)
    s = np.ascontiguousarray(buf[:, D:D + 4]).view(np.float32)
    return q * s + x_inp.astype(np.float32)
